# revision 17
# baseline (speedup 1.0000x reference)
"""Causal self-attention (B=2, T=2048, C=1024, H=16, D=64) on 8 TRN2 cores.

Sharding: data-parallel over batch (4 cores per batch element) x tensor-
parallel over heads (4 heads per core, as 2 pairs stacked on the 128
partitions). Per core: QKV projection for its head slice, causal attention in
a transposed dataflow (S^T kept as [k, q] so PV contracts over full
128-partition k chunks), row-parallel output projection; the 4 partial
projection outputs per batch are summed on the host, plus the bias.

Perf structure (v2):
- Score matmuls have K=D=64: the two heads of a pair are issued as two
  concurrent PE row-tiles (lhsT/rhs at base partitions 0/64 -> auto
  tile_position (0,0)/(64,0)), so scores run at full array rate.
- Diagonal k-chunks are N-restricted: chunk m of a q-block only computes
  columns >= 128m (the rest is fully masked). The exp output is stored
  column-shifted so the per-pair mask multiply is one [128,4,2,128] DVE op
  on the leading triangle.
- Softmax denominators ride as a ones-column in V (PV out M=65, free);
  reciprocal via reciprocal_approx_fast; the per-q recip row is broadcast
  across partitions with two concurrent rank-1 PE matmuls at tile positions
  (0,0)/(64,64).
- ACT (exp) is the co-critical engine (~82us of exp at 153G elem/s vs
  ~100us of PE work). The causal structure back-loads exp work, so q for
  block 3 is computed right after block 0's QKV and jq=3's off-diagonal
  scores are emitted early, unlocked k-block by k-block. A pending-thunk
  queue interleaves QKV/PV/proj matmuls between score pairs so the PE
  never idles while ACT chews exps. PV chains consume diagonal probs first
  so the shared diag tiles recycle quickly.
- All matmul operands bf16 (FWL), accumulation fp32 in PSUM; q weights and
  bias pre-scaled by 1/sqrt(D); no max-subtraction in softmax (scores are
  O(1) for this input scale); masked-out entries are multiplied by 0 after
  exp. Output partials stored bf16.
"""

import numpy as np
import ml_dtypes

import concourse.bass as bass
import concourse.mybir as mybir
import concourse.tile as tile
from concourse import bacc
from concourse.bass_utils import run_bass_kernel_spmd

# Problem shape (hardcoded per contract)
B, T, C, H, D = 2, 2048, 1024, 16, 64
N_CORES = 8
P = 128            # partitions
TB = 512           # q-block width
NTB = T // TB      # 4 q-blocks
NT = T // P        # 16 t-tiles
NC_C = C // P      # 8 contraction chunks over C
HL = 4             # heads per core
CL = HL * D        # 256 local channels
F32 = mybir.dt.float32
BF16 = mybir.dt.bfloat16
NP_BF16 = ml_dtypes.bfloat16
EXP = mybir.ActivationFunctionType.Exp
MUL = mybir.AluOpType.mult
ADD = mybir.AluOpType.add

_CACHE = {}


def _build():
    if "nc" in _CACHE:
        return _CACHE["nc"]
    nc = bacc.Bacc("TRN2", target_bir_lowering=False, debug=False,
                   num_devices=N_CORES)

    xt_d = nc.declare_dram_parameter("xt", [NTB, P, NC_C, TB], BF16, isOutput=False)
    wq_d = nc.declare_dram_parameter("wq", [P, NC_C, CL], BF16, isOutput=False)
    wk_d = nc.declare_dram_parameter("wk", [P, NC_C, CL], BF16, isOutput=False)
    wv_d = nc.declare_dram_parameter("wv", [P, NC_C, CL], BF16, isOutput=False)
    bq_d = nc.declare_dram_parameter("bq", [P, 2], F32, isOutput=False)
    bk_d = nc.declare_dram_parameter("bk", [P, 2], F32, isOutput=False)
    bv_d = nc.declare_dram_parameter("bv", [P, CL], F32, isOutput=False)
    wp_d = nc.declare_dram_parameter("wp", [P, 2, C], BF16, isOutput=False)
    tri_d = nc.declare_dram_parameter("tri", [P, 4, 2, P], BF16, isOutput=False)
    o_d = nc.declare_dram_parameter("o", [NT, P, C], BF16, isOutput=True)

    with tile.TileContext(nc) as tc:
        with (
            tc.tile_pool(name="const", bufs=1) as cw,
            tc.tile_pool(name="xt", bufs=2) as xt_pool,
            tc.tile_pool(name="qkv", bufs=1) as qkv_pool,
            tc.tile_pool(name="pt", bufs=20) as pt_pool,
            tc.tile_pool(name="ptd", bufs=3) as ptd_pool,
            tc.tile_pool(name="stage", bufs=3) as stage_pool,
            tc.tile_pool(name="pA", bufs=2, space="PSUM") as pA,   # 2 banks/tile
            tc.tile_pool(name="pB", bufs=3, space="PSUM") as pB,   # 1 bank/tile
            tc.tile_pool(name="pC", bufs=1, space="PSUM") as pC,   # 1 bank/tile
        ):
            # --- persistent SBUF tensors; DMA in priority order ---
            wq_sb = cw.tile([P, NC_C, CL], BF16)
            wk_sb = cw.tile([P, NC_C, CL], BF16)
            wv_sb = cw.tile([P, NC_C, CL], BF16)
            bq_sb = cw.tile([P, 2], F32)
            bk_sb = cw.tile([P, 2], F32)
            bv_sb = cw.tile([P, CL], F32)
            wp_sb = cw.tile([P, 2, C], BF16)
            tri_sb = cw.tile([P, 4, 2, P], BF16)
            xt3_sb = cw.tile([P, NC_C, TB], BF16)
            dum_sb = cw.tile([1, 8], F32)
            xt_t = [None] * 3

            # wq/xt0 split per 2 c-chunks so QKV0's first chain streams
            # behind the DMA instead of waiting for the full tensors
            xt_t[0] = xt_pool.tile([P, NC_C, TB], BF16, tag="xt", name="xt0")
            for c in range(0, NC_C, 2):
                nc.sync.dma_start(wq_sb[:, c:c + 2, :], wq_d[:, c:c + 2, :])
                nc.sync.dma_start(xt_t[0][:, c:c + 2, :], xt_d[0, :, c:c + 2, :])
            nc.sync.dma_start(bq_sb[:], bq_d[:])
            nc.sync.dma_start(wk_sb[:], wk_d[:])
            nc.sync.dma_start(bk_sb[:], bk_d[:])
            nc.sync.dma_start(xt3_sb[:], xt_d[3])
            nc.sync.dma_start(wv_sb[:], wv_d[:])
            nc.sync.dma_start(bv_sb[:], bv_d[:])
            nc.sync.dma_start(tri_sb[:], tri_d[:])
            xt_t[1] = xt_pool.tile([P, NC_C, TB], BF16, tag="xt", name="xt1")
            nc.sync.dma_start(xt_t[1][:], xt_d[1])
            nc.sync.dma_start(wp_sb[:], wp_d[:])

            # preload the exp table set while DMAs stream
            nc.vector.memset(dum_sb[:], 1.0)
            nc.scalar.activation(dum_sb[:], dum_sb[:], EXP)

            # PE warm-up: ~5us of dummy matmuls on memset data while the
            # input DMAs stream, so QKV0 starts at the un-throttled clock
            warm_sb = cw.tile([P, P], BF16)
            nc.vector.memset(warm_sb[:], 0.0)
            wps = pA.tile([P, 2, TB], F32, tag="pA", name="wps")
            for i in range(48):
                nc.tensor.matmul(wps[:, 0, 0:P], warm_sb[:], warm_sb[:],
                                 start=True, stop=True, skip_group_check=True)

            # qT/kT: [128 = 2 heads x 64d, T]; mt=0 -> heads 0,1; 1 -> 2,3
            q_sb = [qkv_pool.tile([P, T], BF16, tag=f"q{m}", name=f"q{m}")
                    for m in range(2)]
            k_sb = [qkv_pool.tile([P, T], BF16, tag=f"k{m}", name=f"k{m}")
                    for m in range(2)]
            a_sb = [qkv_pool.tile([P, T], BF16, tag=f"a{m}", name=f"a{m}")
                    for m in range(2)]
            # V (+ ones column): [p(k within chunk), t-tile, head, 65]
            v_sb = qkv_pool.tile([P, NT, HL, D + 1], BF16, tag="v")
            nc.vector.memset(v_sb[:, :, :, D:D + 1], 1.0)
            # jq=3 off-diagonal probs, alive until PV3: [chunk, hh, q]
            j3pt = [qkv_pool.tile([P, 12, 2, TB], BF16, tag=f"j3{m}",
                                  name=f"j3{m}") for m in range(2)]
            # rank-1 broadcast lhsT: ones at partition rows 0..64
            ones_sb = cw.tile([P // 2 + 1, D], F32)
            nc.vector.memset(ones_sb[:], 1.0)

            # ---------- pending-thunk machinery ----------
            pending = []      # list of (cost_ns, fn)
            n_popped = [0]

            def drain(budget_ns):
                while pending and budget_ns > 0:
                    cost, fn = pending.pop(0)
                    fn()
                    n_popped[0] += 1
                    budget_ns -= cost

            def mark():
                return n_popped[0] + len(pending)

            def flush_upto(m):
                while n_popped[0] < m:
                    _, fn = pending.pop(0)
                    fn()
                    n_popped[0] += 1

            def flush_all():
                while pending:
                    _, fn = pending.pop(0)
                    fn()
                    n_popped[0] += 1

            # ---------- QKV building blocks ----------
            def qk_chain(jt, mt, xsrc):
                def fn(jt=jt, mt=mt, xsrc=xsrc):
                    tsl = bass.ts(jt, TB)
                    msl = bass.ts(mt, P)
                    pqk = pA.tile([P, 2, TB], F32, tag="pA", name="pqk")
                    for c in range(NC_C):
                        nc.tensor.matmul(pqk[:, 0, :], wq_sb[:, c, msl],
                                         xsrc[:, c, :],
                                         start=(c == 0), stop=(c == NC_C - 1),
                                         skip_group_check=True)
                    for c in range(NC_C):
                        nc.tensor.matmul(pqk[:, 1, :], wk_sb[:, c, msl],
                                         xsrc[:, c, :],
                                         start=(c == 0), stop=(c == NC_C - 1),
                                         skip_group_check=True)
                    nc.vector.tensor_scalar_add(q_sb[mt][:, tsl], pqk[:, 0, :],
                                                bq_sb[:, mt:mt + 1])
                    nc.vector.tensor_scalar_add(k_sb[mt][:, tsl], pqk[:, 1, :],
                                                bk_sb[:, mt:mt + 1])
                return (3700, fn)

            def q_chain(jt, mt, xsrc):
                def fn(jt=jt, mt=mt, xsrc=xsrc):
                    pq = pA.tile([P, 2, TB], F32, tag="pA", name="pq")
                    for c in range(NC_C):
                        nc.tensor.matmul(pq[:, 0, :], wq_sb[:, c, bass.ts(mt, P)],
                                         xsrc[:, c, :],
                                         start=(c == 0), stop=(c == NC_C - 1),
                                         skip_group_check=True)
                    nc.vector.tensor_scalar_add(q_sb[mt][:, bass.ts(jt, TB)],
                                                pq[:, 0, :], bq_sb[:, mt:mt + 1])
                return (1850, fn)

            def k_chain(jt, mt, xsrc):
                def fn(jt=jt, mt=mt, xsrc=xsrc):
                    pk = pA.tile([P, 2, TB], F32, tag="pA", name="pk")
                    for c in range(NC_C):
                        nc.tensor.matmul(pk[:, 0, :], wk_sb[:, c, bass.ts(mt, P)],
                                         xsrc[:, c, :],
                                         start=(c == 0), stop=(c == NC_C - 1),
                                         skip_group_check=True)
                    nc.vector.tensor_scalar_add(k_sb[mt][:, bass.ts(jt, TB)],
                                                pk[:, 0, :], bk_sb[:, mt:mt + 1])
                return (1850, fn)

            def v_chain(jt, t4, xsrc):
                def fn(jt=jt, t4=t4, xsrc=xsrc):
                    tt = NTB * jt + t4
                    psv_t = pA.tile([P, 2, TB], F32, tag="pA", name="psv")
                    psv = psv_t[:, 0, 0:CL]
                    for c in range(NC_C):
                        nc.tensor.matmul(psv, xsrc[:, c, bass.ts(t4, P)],
                                         wv_sb[:, c, :],
                                         start=(c == 0), stop=(c == NC_C - 1),
                                         skip_group_check=True)
                    nc.vector.tensor_tensor(
                        v_sb[:, tt, :, 0:D],
                        psv.rearrange("p (h d) -> p h d", h=HL),
                        bv_sb[:].rearrange("p (h d) -> p h d", h=HL),
                        ADD)
                return (2050, fn)

            # ---------- attention building blocks ----------
            ptd_of = {}

            def score_diag(jq, mt, budget):
                """4 diagonal chunk-pairs + triangle mask for (jq, mt).
                Chunk m computes cols [128m, 512); exp stored column-shifted
                (ptd col c = q - 128m) so the mask is one leading-triangle
                multiply."""
                ptd = ptd_pool.tile([P, NTB, 2, TB], BF16, tag="ptd",
                                    name="ptd")
                ptd_of[(jq, mt)] = ptd
                for m in range(NTB):
                    ik = NTB * jq + m
                    w = TB - P * m
                    ps2 = pA.tile([P, 2, TB], F32, tag="pA", name="ps2")
                    for hh in range(2):
                        hsl = bass.ts(hh, D)
                        nc.tensor.matmul(
                            ps2[:, hh, P * m:TB],
                            k_sb[mt][hsl, bass.ts(ik, P)],
                            q_sb[mt][hsl, TB * jq + P * m:TB * (jq + 1)],
                            start=True, stop=True, skip_group_check=True)
                    nc.scalar.activation(ptd[:, m, :, 0:w],
                                         ps2[:, :, P * m:TB], EXP)
                    drain(budget)
                nc.vector.tensor_tensor(ptd[:, :, :, 0:P], ptd[:, :, :, 0:P],
                                        tri_sb[:], MUL)

            def score_off(jq, mt, ik, out_ap, budget):
                ps2 = pA.tile([P, 2, TB], F32, tag="pA", name="ps2")
                for hh in range(2):
                    hsl = bass.ts(hh, D)
                    nc.tensor.matmul(ps2[:, hh, :],
                                     k_sb[mt][hsl, bass.ts(ik, P)],
                                     q_sb[mt][hsl, bass.ts(jq, TB)],
                                     start=True, stop=True,
                                     skip_group_check=True)
                nc.scalar.activation(out_ap, ps2[:], EXP)
                drain(budget)

            pa_state = {}

            def queue_pv(jq, mt, pts_of, part="all"):
                """PV chains for both heads of pair mt. diag chunks first
                (frees the shared ptd tile early). part="off"/"diag" splits
                jq3's chain so the long off part pre-drains as soon as its
                exps exist, before the diagonal exps are even emitted."""
                pa2 = pa_state.setdefault((jq, mt), [None, None])
                offs = list(range(NTB * jq))
                order = []
                if part == "all_off_first":
                    order += [("o", ik) for ik in offs]
                    order += [("d", m) for m in range(NTB)]
                    starts, ends = ("o", 0), order[-1]
                elif part in ("all",):
                    order += [("d", m) for m in range(NTB)]
                    order += [("o", ik) for ik in offs]
                    starts, ends = ("d", 0), order[-1]
                elif part == "off":
                    order += [("o", ik) for ik in offs]
                    starts, ends = ("o", 0), (None, None)
                else:  # diag tail after an off part
                    order += [("d", m) for m in range(NTB)]
                    starts, ends = (None, None), ("d", NTB - 1)
                for i, (kind, idx) in enumerate(order):
                    first = (kind, idx) == starts
                    last = (kind, idx) == ends
                    for hh in range(2):
                        if kind == "d":
                            m = idx
                            def mm(jq=jq, mt=mt, m=m, hh=hh, w=TB - P * m,
                                   first=first, last=last):
                                if first:
                                    pa2[hh] = pB.tile([D + 1, TB], F32,
                                                      tag="pB", name="pa")
                                nc.tensor.matmul(pa2[hh][:, TB - w:TB],
                                                 v_sb[:, NTB * jq + m,
                                                      2 * mt + hh, :],
                                                 ptd_of[(jq, mt)][:, m, hh, 0:w],
                                                 start=first, stop=last,
                                                 skip_group_check=True)
                        else:
                            ik = idx
                            def mm(mt=mt, ik=ik, hh=hh, pts_of=pts_of,
                                   first=first, last=last):
                                if first:
                                    pa2[hh] = pB.tile([D + 1, TB], F32,
                                                      tag="pB", name="pa")
                                nc.tensor.matmul(pa2[hh][:],
                                                 v_sb[:, ik, 2 * mt + hh, :],
                                                 pts_of(ik, hh),
                                                 start=first, stop=last,
                                                 skip_group_check=True)
                        pending.append((430, mm))

                def norm(jq=jq, mt=mt):
                    qsl = bass.ts(jq, TB)
                    ua = stage_pool.tile([P, TB], F32, tag="ua", name="ua",
                                         bufs=2)
                    dn = stage_pool.tile([P // 2 + 1, TB], F32, tag="dn",
                                         name="dn", bufs=2)
                    rc = stage_pool.tile([P // 2 + 1, TB], F32, tag="rc",
                                         name="rc", bufs=2)
                    for hh in range(2):
                        nc.vector.tensor_copy(ua[64 * hh:64 * hh + D, :],
                                              pa2[hh][0:D, :])
                        nc.vector.tensor_copy(dn[64 * hh:64 * hh + 1, :],
                                              pa2[hh][D:D + 1, :])
                    nc.vector.reciprocal_approx_fast(rc[:], dn[:])
                    bcp_t = pC.tile([P, TB], F32, tag="pC", name="bcp")
                    bcp = bcp_t[:]
                    for hh in range(2):
                        nc.tensor.matmul(bcp[64 * hh:64 * hh + D, :],
                                         ones_sb[64 * hh:64 * hh + 1, :],
                                         rc[64 * hh:64 * hh + 1, :],
                                         start=True, stop=True,
                                         skip_group_check=True)
                    nc.vector.tensor_tensor(a_sb[mt][:, qsl], ua[:], bcp,
                                            MUL)
                if part != "off":
                    pending.append((900, norm))

            def queue_proj(jq):
                for t4 in range(NTB):
                    def fn(jq=jq, t4=t4):
                        tt = NTB * jq + t4
                        pso = pA.tile([P, 2, TB], F32, tag="pA", name="pso")
                        for nt in range(2):
                            for c2 in range(2):
                                nc.tensor.matmul(
                                    pso[:, nt, :],
                                    a_sb[c2][:, bass.ts(tt, P)],
                                    wp_sb[:, c2, bass.ts(nt, TB)],
                                    start=(c2 == 0), stop=(c2 == 1),
                                    skip_group_check=True)
                        st = stage_pool.tile([P, 2 * TB], BF16, tag="st",
                                             name="st", bufs=2)
                        nc.vector.tensor_copy(
                            st[:].rearrange("p (n f) -> p n f", n=2), pso[:])
                        nc.sync.dma_start(o_d[tt], st[:])
                    pending.append((1900, fn))

            # ================= emission =================
            # First qk chain inline, then scores start immediately so ACT
            # (the co-critical engine) ramps as early as possible; the rest
            # of block 0 + q3 drain as fillers between ACT-paced pairs.
            qk_chain(0, 0, xt_t[0])[1]()
            pending.append(qk_chain(0, 1, xt_t[0]))
            for t4 in range(NTB):
                pending.append(v_chain(0, t4, xt_t[0]))
            pending.append(q_chain(3, 0, xt3_sb))
            pending.append(q_chain(3, 1, xt3_sb))

            score_diag(0, 0, 500)
            score_diag(0, 1, 500)
            flush_all()

            # PV0 + B0 first: PV's diag part frees ptd(0) tiles, which
            # S(jq1)'s diagonal exps need for their ptd allocations
            for mt in range(2):
                queue_pv(0, mt, None)
            for t4 in range(NTB):
                pending.append(v_chain(1, t4, xt_t[1]))

            # S(jq3, k-chunks 0-3)
            for ik in range(0, 4):
                for mt in range(2):
                    score_off(3, mt, ik, j3pt[mt][:, ik, :, :], 900)

            # prefetch xt2 (slot freed once xt0 consumed)
            xt_t[2] = xt_pool.tile([P, NC_C, TB], BF16, tag="xt", name="xt2")
            nc.sync.dma_start(xt_t[2][:], xt_d[2])

            # S(jq1): qk chains inline per mt, right before the scores that
            # need them (the drains keep ACT fed with the other mt's exps)
            pt_of = {}
            qk_chain(1, 0, xt_t[1])[1]()
            score_diag(1, 0, 500)
            qk_chain(1, 1, xt_t[1])[1]()
            score_diag(1, 1, 500)
            for ik in range(0, 4):
                for mt in range(2):
                    ptt = pt_pool.tile([P, 2, TB], BF16, tag="pt", name="pt")
                    pt_of[(1, mt, ik)] = ptt
                    score_off(1, mt, ik, ptt[:], 850)

            # PV1 + B1 first (frees ptd(1) for S(jq2)'s diag)
            for mt in range(2):
                queue_pv(1, mt,
                         lambda ik, hh, mt=mt: pt_of[(1, mt, ik)][:, hh, :])
            for t4 in range(NTB):
                pending.append(v_chain(2, t4, xt_t[2]))
            queue_proj(0)

            # S(jq3, k-chunks 4-7)
            for ik in range(4, 8):
                for mt in range(2):
                    score_off(3, mt, ik, j3pt[mt][:, ik, :, :], 900)

            # S(jq2): qk chains inline per mt
            qk_chain(2, 0, xt_t[2])[1]()
            score_diag(2, 0, 500)
            qk_chain(2, 1, xt_t[2])[1]()
            score_diag(2, 1, 500)
            for ik in range(0, 8):
                for mt in range(2):
                    ptt = pt_pool.tile([P, 2, TB], BF16, tag="pt", name="pt")
                    pt_of[(2, mt, ik)] = ptt
                    score_off(2, mt, ik, ptt[:], 900)

            # PV2 + B2 first (frees ptd(2) for S(jq3)'s diag)
            for mt in range(2):
                queue_pv(2, mt,
                         lambda ik, hh, mt=mt: pt_of[(2, mt, ik)][:, hh, :])
            for t4 in range(NTB):
                pending.append(v_chain(3, t4, xt3_sb))
            queue_proj(1)

            # S(jq3, k-chunks 8-11)
            for ik in range(8, 12):
                for mt in range(2):
                    score_off(3, mt, ik, j3pt[mt][:, ik, :, :], 1100)

            # S(jq3 diag): k3 chains inline per mt; last exps
            k_chain(3, 0, xt3_sb)[1]()
            score_diag(3, 0, 1400)
            k_chain(3, 1, xt3_sb)[1]()
            score_diag(3, 1, 1400)

            # PV3 full chains per pair, off part first (those exps are all
            # done, so the chains stream without ACT waits)
            for mt in range(2):
                queue_pv(3, mt,
                         lambda ik, hh, mt=mt: j3pt[mt][:, ik, hh, :],
                         part="all_off_first")
            queue_proj(2)
            flush_all()
            queue_proj(3)
            flush_all()

    nc.compile()
    _CACHE["nc"] = nc
    return nc


def _prep_core_inputs(x, w_attn, b_attn, w_proj, c):
    b, hg = divmod(c, 4)
    cs = slice(CL * hg, CL * (hg + 1))  # this core's 256 channels
    scale = np.float32(1.0 / np.sqrt(D))

    xt = np.ascontiguousarray(
        x[b].reshape(NTB, TB, NC_C, P).transpose(0, 3, 2, 1)).astype(NP_BF16)
    wq = np.ascontiguousarray(
        (w_attn[:, cs] * scale).reshape(NC_C, P, CL).transpose(1, 0, 2)
    ).astype(NP_BF16)
    wk = np.ascontiguousarray(
        w_attn[:, C:][:, cs].reshape(NC_C, P, CL).transpose(1, 0, 2)
    ).astype(NP_BF16)
    wv = np.ascontiguousarray(
        w_attn[:, 2 * C:][:, cs].reshape(NC_C, P, CL).transpose(1, 0, 2)
    ).astype(NP_BF16)
    bq = np.ascontiguousarray((b_attn[cs] * scale).reshape(2, P).T)
    bk = np.ascontiguousarray(b_attn[C:][cs].reshape(2, P).T)
    bv = np.ascontiguousarray(np.broadcast_to(b_attn[2 * C:][cs], (P, CL)))
    wp = np.ascontiguousarray(
        w_proj[cs, :].reshape(2, P, C).transpose(1, 0, 2)).astype(NP_BF16)

    p_idx = np.arange(P)[:, None, None, None]
    col = np.arange(P)[None, None, None, :]
    tri = np.ascontiguousarray(
        np.broadcast_to((col >= p_idx), (P, 4, 2, P))).astype(NP_BF16)

    return {"xt": xt, "wq": wq, "wk": wk, "wv": wv, "bq": bq, "bk": bk,
            "bv": bv, "wp": wp, "tri": tri}


def kernel(x, w_attn, b_attn, w_proj, b_proj):
    x = np.asarray(x, dtype=np.float32)
    w_attn = np.asarray(w_attn, dtype=np.float32)
    b_attn = np.asarray(b_attn, dtype=np.float32)
    w_proj = np.asarray(w_proj, dtype=np.float32)
    b_proj = np.asarray(b_proj, dtype=np.float32)

    nc = _build()
    in_maps = [_prep_core_inputs(x, w_attn, b_attn, w_proj, c)
               for c in range(N_CORES)]
    res = run_bass_kernel_spmd(nc, in_maps, list(range(N_CORES)))

    out = np.empty((B, T, C), dtype=np.float32)
    for b in range(B):
        acc = np.zeros((T, C), dtype=np.float32)
        for c in range(4 * b, 4 * b + 4):
            acc += res.results[c]["o"].astype(np.float32).reshape(T, C)
        out[b] = acc + b_proj
    return out


# revision 19
# speedup vs baseline: 1.0574x; 1.0574x over previous
"""Causal self-attention (B=2, T=2048, C=1024, H=16, D=64) on 8 TRN2 cores.

Sharding: data-parallel over batch (4 cores per batch element) x tensor-
parallel over heads (4 heads per core, as 2 pairs stacked on the 128
partitions). Per core: QKV projection for its head slice, causal attention in
a transposed dataflow (S^T kept as [k, q] so PV contracts over full
128-partition k chunks), row-parallel output projection; the 4 partial
projection outputs per batch are summed on the host, plus the bias.

Perf structure (v2):
- Score matmuls have K=D=64: the two heads of a pair are issued as two
  concurrent PE row-tiles (lhsT/rhs at base partitions 0/64 -> auto
  tile_position (0,0)/(64,0)), so scores run at full array rate.
- Diagonal k-chunks are N-restricted: chunk m of a q-block only computes
  columns >= 128m (the rest is fully masked). The exp output is stored
  column-shifted so the per-pair mask multiply is one [128,4,2,128] DVE op
  on the leading triangle.
- Softmax denominators ride as a ones-column in V (PV out M=65, free);
  reciprocal via reciprocal_approx_fast; the per-q recip row is broadcast
  across partitions with two concurrent rank-1 PE matmuls at tile positions
  (0,0)/(64,64).
- ACT (exp) is the co-critical engine (~82us of exp at 153G elem/s vs
  ~100us of PE work). The causal structure back-loads exp work, so q for
  block 3 is computed right after block 0's QKV and jq=3's off-diagonal
  scores are emitted early, unlocked k-block by k-block. A pending-thunk
  queue interleaves QKV/PV/proj matmuls between score pairs so the PE
  never idles while ACT chews exps. PV chains consume diagonal probs first
  so the shared diag tiles recycle quickly.
- All matmul operands bf16 (FWL), accumulation fp32 in PSUM; q weights and
  bias pre-scaled by 1/sqrt(D); no max-subtraction in softmax (scores are
  O(1) for this input scale); masked-out entries are multiplied by 0 after
  exp. Output partials stored bf16.
"""

import numpy as np
import ml_dtypes

import concourse.bass as bass
import concourse.mybir as mybir
import concourse.tile as tile
from concourse import bacc
from concourse.bass_utils import run_bass_kernel_spmd

# Problem shape (hardcoded per contract)
B, T, C, H, D = 2, 2048, 1024, 16, 64
N_CORES = 8
P = 128            # partitions
TB = 512           # q-block width
NTB = T // TB      # 4 q-blocks
NT = T // P        # 16 t-tiles
NC_C = C // P      # 8 contraction chunks over C
HL = 4             # heads per core
CL = HL * D        # 256 local channels
F32 = mybir.dt.float32
BF16 = mybir.dt.bfloat16
NP_BF16 = ml_dtypes.bfloat16
EXP = mybir.ActivationFunctionType.Exp
MUL = mybir.AluOpType.mult
ADD = mybir.AluOpType.add

_CACHE = {}


def _build():
    if "nc" in _CACHE:
        return _CACHE["nc"]
    nc = bacc.Bacc("TRN2", target_bir_lowering=False, debug=False,
                   num_devices=N_CORES)

    xt_d = nc.declare_dram_parameter("xt", [NTB, P, NC_C, TB], BF16, isOutput=False)
    wq_d = nc.declare_dram_parameter("wq", [P, NC_C, CL], BF16, isOutput=False)
    wk_d = nc.declare_dram_parameter("wk", [P, NC_C, CL], BF16, isOutput=False)
    wv_d = nc.declare_dram_parameter("wv", [P, NC_C, CL], BF16, isOutput=False)
    bq_d = nc.declare_dram_parameter("bq", [P, 2], F32, isOutput=False)
    bk_d = nc.declare_dram_parameter("bk", [P, 2], F32, isOutput=False)
    bv_d = nc.declare_dram_parameter("bv", [P, CL], F32, isOutput=False)
    wp_d = nc.declare_dram_parameter("wp", [P, 2, C], BF16, isOutput=False)
    tri_d = nc.declare_dram_parameter("tri", [P, 4, 2, P], BF16, isOutput=False)
    o_d = nc.declare_dram_parameter("o", [NT, P, C], BF16, isOutput=True)

    with tile.TileContext(nc) as tc:
        with (
            tc.tile_pool(name="const", bufs=1) as cw,
            tc.tile_pool(name="xt", bufs=2) as xt_pool,
            tc.tile_pool(name="qkv", bufs=1) as qkv_pool,
            tc.tile_pool(name="pt", bufs=20) as pt_pool,
            tc.tile_pool(name="ptd", bufs=3) as ptd_pool,
            tc.tile_pool(name="stage", bufs=3) as stage_pool,
            tc.tile_pool(name="pA", bufs=2, space="PSUM") as pA,   # 2 banks/tile
            tc.tile_pool(name="pB", bufs=3, space="PSUM") as pB,   # 1 bank/tile
            tc.tile_pool(name="pC", bufs=1, space="PSUM") as pC,   # 1 bank/tile
        ):
            # --- persistent SBUF tensors; DMA in priority order ---
            wq_sb = cw.tile([P, NC_C, CL], BF16)
            wk_sb = cw.tile([P, NC_C, CL], BF16)
            wv_sb = cw.tile([P, NC_C, CL], BF16)
            bq_sb = cw.tile([P, 2], F32)
            bk_sb = cw.tile([P, 2], F32)
            bv_sb = cw.tile([P, CL], F32)
            wp_sb = cw.tile([P, 2, C], BF16)
            tri_sb = cw.tile([P, 4, 2, P], BF16)
            xt3_sb = cw.tile([P, NC_C, TB], BF16)
            dum_sb = cw.tile([1, 8], F32)
            xt_t = [None] * 3

            # wq/xt0 split per 2 c-chunks so QKV0's first chain streams
            # behind the DMA instead of waiting for the full tensors
            xt_t[0] = xt_pool.tile([P, NC_C, TB], BF16, tag="xt", name="xt0")
            for c in range(0, NC_C, 2):
                nc.sync.dma_start(wq_sb[:, c:c + 2, :], wq_d[:, c:c + 2, :])
                nc.sync.dma_start(xt_t[0][:, c:c + 2, :], xt_d[0, :, c:c + 2, :])
            nc.sync.dma_start(bq_sb[:], bq_d[:])
            nc.sync.dma_start(wk_sb[:], wk_d[:])
            nc.sync.dma_start(bk_sb[:], bk_d[:])
            nc.sync.dma_start(xt3_sb[:], xt_d[3])
            nc.sync.dma_start(wv_sb[:], wv_d[:])
            nc.sync.dma_start(bv_sb[:], bv_d[:])
            nc.sync.dma_start(tri_sb[:], tri_d[:])
            xt_t[1] = xt_pool.tile([P, NC_C, TB], BF16, tag="xt", name="xt1")
            nc.sync.dma_start(xt_t[1][:], xt_d[1])
            nc.sync.dma_start(wp_sb[:], wp_d[:])

            # preload the exp table set while DMAs stream
            nc.vector.memset(dum_sb[:], 1.0)
            nc.scalar.activation(dum_sb[:], dum_sb[:], EXP)

            # PE warm-up: ~5us of dummy matmuls on memset data while the
            # input DMAs stream, so QKV0 starts at the un-throttled clock
            warm_sb = cw.tile([P, P], BF16)
            nc.vector.memset(warm_sb[:], 0.0)
            wps = pA.tile([P, 2, TB], F32, tag="pA", name="wps")
            for i in range(48):
                nc.tensor.matmul(wps[:, 0, 0:P], warm_sb[:], warm_sb[:],
                                 start=True, stop=True, skip_group_check=True)

            # qT/kT: [128 = 2 heads x 64d, T]; mt=0 -> heads 0,1; 1 -> 2,3
            q_sb = [qkv_pool.tile([P, T], BF16, tag=f"q{m}", name=f"q{m}")
                    for m in range(2)]
            k_sb = [qkv_pool.tile([P, T], BF16, tag=f"k{m}", name=f"k{m}")
                    for m in range(2)]
            a_sb = [qkv_pool.tile([P, T], BF16, tag=f"a{m}", name=f"a{m}")
                    for m in range(2)]
            # V (+ ones column): [p(k within chunk), t-tile, head, 65]
            v_sb = qkv_pool.tile([P, NT, HL, D + 1], BF16, tag="v")
            nc.vector.memset(v_sb[:, :, :, D:D + 1], 1.0)
            # jq=3 off-diagonal probs, alive until PV3: [chunk, hh, q]
            j3pt = [qkv_pool.tile([P, 12, 2, TB], BF16, tag=f"j3{m}",
                                  name=f"j3{m}") for m in range(2)]
            # rank-1 broadcast lhsT: ones at partition rows 0..64
            ones_sb = cw.tile([P // 2 + 1, D], F32)
            nc.vector.memset(ones_sb[:], 1.0)

            # ---------- pending-thunk machinery ----------
            pending = []      # list of (cost_ns, fn)
            n_popped = [0]

            def drain(budget_ns):
                while pending and budget_ns > 0:
                    cost, fn = pending.pop(0)
                    fn()
                    n_popped[0] += 1
                    budget_ns -= cost

            def mark():
                return n_popped[0] + len(pending)

            def flush_upto(m):
                while n_popped[0] < m:
                    _, fn = pending.pop(0)
                    fn()
                    n_popped[0] += 1

            def flush_all():
                while pending:
                    _, fn = pending.pop(0)
                    fn()
                    n_popped[0] += 1

            # ---------- QKV building blocks ----------
            def qk_chain(jt, mt, xsrc):
                def fn(jt=jt, mt=mt, xsrc=xsrc):
                    tsl = bass.ts(jt, TB)
                    msl = bass.ts(mt, P)
                    pqk = pA.tile([P, 2, TB], F32, tag="pA", name="pqk")
                    for c in range(NC_C):
                        nc.tensor.matmul(pqk[:, 0, :], wq_sb[:, c, msl],
                                         xsrc[:, c, :],
                                         start=(c == 0), stop=(c == NC_C - 1),
                                         skip_group_check=True)
                    for c in range(NC_C):
                        nc.tensor.matmul(pqk[:, 1, :], wk_sb[:, c, msl],
                                         xsrc[:, c, :],
                                         start=(c == 0), stop=(c == NC_C - 1),
                                         skip_group_check=True)
                    nc.vector.tensor_scalar_add(q_sb[mt][:, tsl], pqk[:, 0, :],
                                                bq_sb[:, mt:mt + 1])
                    nc.vector.tensor_scalar_add(k_sb[mt][:, tsl], pqk[:, 1, :],
                                                bk_sb[:, mt:mt + 1])
                return (3700, fn)

            def q_chain(jt, mt, xsrc):
                def fn(jt=jt, mt=mt, xsrc=xsrc):
                    pq = pA.tile([P, 2, TB], F32, tag="pA", name="pq")
                    for c in range(NC_C):
                        nc.tensor.matmul(pq[:, 0, :], wq_sb[:, c, bass.ts(mt, P)],
                                         xsrc[:, c, :],
                                         start=(c == 0), stop=(c == NC_C - 1),
                                         skip_group_check=True)
                    nc.vector.tensor_scalar_add(q_sb[mt][:, bass.ts(jt, TB)],
                                                pq[:, 0, :], bq_sb[:, mt:mt + 1])
                return (1850, fn)

            def k_chain(jt, mt, xsrc):
                def fn(jt=jt, mt=mt, xsrc=xsrc):
                    pk = pA.tile([P, 2, TB], F32, tag="pA", name="pk")
                    for c in range(NC_C):
                        nc.tensor.matmul(pk[:, 0, :], wk_sb[:, c, bass.ts(mt, P)],
                                         xsrc[:, c, :],
                                         start=(c == 0), stop=(c == NC_C - 1),
                                         skip_group_check=True)
                    nc.vector.tensor_scalar_add(k_sb[mt][:, bass.ts(jt, TB)],
                                                pk[:, 0, :], bk_sb[:, mt:mt + 1])
                return (1850, fn)

            def v_chain(jt, t4, xsrc):
                def fn(jt=jt, t4=t4, xsrc=xsrc):
                    tt = NTB * jt + t4
                    psv_t = pA.tile([P, 2, TB], F32, tag="pA", name="psv")
                    psv = psv_t[:, 0, 0:CL]
                    for c in range(NC_C):
                        nc.tensor.matmul(psv, xsrc[:, c, bass.ts(t4, P)],
                                         wv_sb[:, c, :],
                                         start=(c == 0), stop=(c == NC_C - 1),
                                         skip_group_check=True)
                    nc.vector.tensor_tensor(
                        v_sb[:, tt, :, 0:D],
                        psv.rearrange("p (h d) -> p h d", h=HL),
                        bv_sb[:].rearrange("p (h d) -> p h d", h=HL),
                        ADD)
                return (2050, fn)

            # ---------- attention building blocks ----------
            ptd_of = {}

            def score_diag(jq, mt, budget):
                """4 diagonal chunk-pairs + triangle mask for (jq, mt).
                Chunk m computes cols [128m, 512); exp stored column-shifted
                (ptd col c = q - 128m) so the mask is one leading-triangle
                multiply."""
                ptd = ptd_pool.tile([P, NTB, 2, TB], BF16, tag="ptd",
                                    name="ptd")
                ptd_of[(jq, mt)] = ptd
                for m in range(NTB):
                    ik = NTB * jq + m
                    w = TB - P * m
                    ps2 = pA.tile([P, 2, TB], F32, tag="pA", name="ps2")
                    for hh in range(2):
                        hsl = bass.ts(hh, D)
                        nc.tensor.matmul(
                            ps2[:, hh, P * m:TB],
                            k_sb[mt][hsl, bass.ts(ik, P)],
                            q_sb[mt][hsl, TB * jq + P * m:TB * (jq + 1)],
                            start=True, stop=True, skip_group_check=True)
                    nc.scalar.activation(ptd[:, m, :, 0:w],
                                         ps2[:, :, P * m:TB], EXP)
                    drain(budget)
                nc.vector.tensor_tensor(ptd[:, :, :, 0:P], ptd[:, :, :, 0:P],
                                        tri_sb[:], MUL)

            def score_off(jq, mt, ik, out_ap, budget):
                ps2 = pA.tile([P, 2, TB], F32, tag="pA", name="ps2")
                for hh in range(2):
                    hsl = bass.ts(hh, D)
                    nc.tensor.matmul(ps2[:, hh, :],
                                     k_sb[mt][hsl, bass.ts(ik, P)],
                                     q_sb[mt][hsl, bass.ts(jq, TB)],
                                     start=True, stop=True,
                                     skip_group_check=True)
                nc.scalar.activation(out_ap, ps2[:], EXP)
                drain(budget)

            pa_state = {}

            def queue_pv(jq, mt, pts_of, part="all"):
                """PV chains for both heads of pair mt. diag chunks first
                (frees the shared ptd tile early). part="off"/"diag" splits
                jq3's chain so the long off part pre-drains as soon as its
                exps exist, before the diagonal exps are even emitted."""
                pa2 = pa_state.setdefault((jq, mt), [None, None])
                offs = list(range(NTB * jq))
                order = []
                if part == "all_off_first":
                    order += [("o", ik) for ik in offs]
                    order += [("d", m) for m in range(NTB)]
                    starts, ends = ("o", 0), order[-1]
                elif part in ("all",):
                    order += [("d", m) for m in range(NTB)]
                    order += [("o", ik) for ik in offs]
                    starts, ends = ("d", 0), order[-1]
                elif part == "off":
                    order += [("o", ik) for ik in offs]
                    starts, ends = ("o", 0), (None, None)
                else:  # diag tail after an off part
                    order += [("d", m) for m in range(NTB)]
                    starts, ends = (None, None), ("d", NTB - 1)
                for i, (kind, idx) in enumerate(order):
                    first = (kind, idx) == starts
                    last = (kind, idx) == ends
                    for hh in range(2):
                        if kind == "d":
                            m = idx
                            def mm(jq=jq, mt=mt, m=m, hh=hh, w=TB - P * m,
                                   first=first, last=last):
                                if first:
                                    pa2[hh] = pB.tile([D + 1, TB], F32,
                                                      tag="pB", name="pa")
                                nc.tensor.matmul(pa2[hh][:, TB - w:TB],
                                                 v_sb[:, NTB * jq + m,
                                                      2 * mt + hh, :],
                                                 ptd_of[(jq, mt)][:, m, hh, 0:w],
                                                 start=first, stop=last,
                                                 skip_group_check=True)
                        else:
                            ik = idx
                            def mm(mt=mt, ik=ik, hh=hh, pts_of=pts_of,
                                   first=first, last=last):
                                if first:
                                    pa2[hh] = pB.tile([D + 1, TB], F32,
                                                      tag="pB", name="pa")
                                nc.tensor.matmul(pa2[hh][:],
                                                 v_sb[:, ik, 2 * mt + hh, :],
                                                 pts_of(ik, hh),
                                                 start=first, stop=last,
                                                 skip_group_check=True)
                        pending.append((430, mm))

                def norm(jq=jq, mt=mt):
                    qsl = bass.ts(jq, TB)
                    ua = stage_pool.tile([P, TB], F32, tag="ua", name="ua",
                                         bufs=2)
                    dn = stage_pool.tile([P // 2 + 1, TB], F32, tag="dn",
                                         name="dn", bufs=2)
                    rc = stage_pool.tile([P // 2 + 1, TB], F32, tag="rc",
                                         name="rc", bufs=2)
                    for hh in range(2):
                        nc.vector.tensor_copy(ua[64 * hh:64 * hh + D, :],
                                              pa2[hh][0:D, :])
                        nc.vector.tensor_copy(dn[64 * hh:64 * hh + 1, :],
                                              pa2[hh][D:D + 1, :])
                    nc.vector.reciprocal_approx_fast(rc[:], dn[:])
                    bcp_t = pC.tile([P, TB], F32, tag="pC", name="bcp")
                    bcp = bcp_t[:]
                    for hh in range(2):
                        nc.tensor.matmul(bcp[64 * hh:64 * hh + D, :],
                                         ones_sb[64 * hh:64 * hh + 1, :],
                                         rc[64 * hh:64 * hh + 1, :],
                                         start=True, stop=True,
                                         skip_group_check=True)
                    nc.vector.tensor_tensor(a_sb[mt][:, qsl], ua[:], bcp,
                                            MUL)
                if part != "off":
                    pending.append((900, norm))

            def queue_proj(jq):
                for t4 in range(NTB):
                    def fn(jq=jq, t4=t4):
                        tt = NTB * jq + t4
                        pso = pA.tile([P, 2, TB], F32, tag="pA", name="pso")
                        for nt in range(2):
                            for c2 in range(2):
                                nc.tensor.matmul(
                                    pso[:, nt, :],
                                    a_sb[c2][:, bass.ts(tt, P)],
                                    wp_sb[:, c2, bass.ts(nt, TB)],
                                    start=(c2 == 0), stop=(c2 == 1),
                                    skip_group_check=True)
                        st = stage_pool.tile([P, 2 * TB], BF16, tag="st",
                                             name="st", bufs=2)
                        nc.vector.tensor_copy(
                            st[:].rearrange("p (n f) -> p n f", n=2), pso[:])
                        nc.sync.dma_start(o_d[tt], st[:])
                    pending.append((1900, fn))

            # ================= emission =================
            # First qk chain inline, then scores start immediately so ACT
            # (the co-critical engine) ramps as early as possible; the rest
            # of block 0 + q3 drain as fillers between ACT-paced pairs.
            qk_chain(0, 0, xt_t[0])[1]()
            pending.append(qk_chain(0, 1, xt_t[0]))
            for t4 in range(NTB):
                pending.append(v_chain(0, t4, xt_t[0]))
            pending.append(q_chain(3, 0, xt3_sb))
            pending.append(q_chain(3, 1, xt3_sb))

            score_diag(0, 0, 500)
            score_diag(0, 1, 500)
            flush_all()

            # PV0 + B0 first: PV's diag part frees ptd(0) tiles, which
            # S(jq1)'s diagonal exps need for their ptd allocations
            for mt in range(2):
                queue_pv(0, mt, None)
            for t4 in range(NTB):
                pending.append(v_chain(1, t4, xt_t[1]))

            # S(jq3, k-chunks 0-3)
            for ik in range(0, 4):
                for mt in range(2):
                    score_off(3, mt, ik, j3pt[mt][:, ik, :, :], 900)

            # prefetch xt2 (slot freed once xt0 consumed)
            xt_t[2] = xt_pool.tile([P, NC_C, TB], BF16, tag="xt", name="xt2")
            nc.sync.dma_start(xt_t[2][:], xt_d[2])

            # S(jq1): qk chains inline per mt, right before the scores that
            # need them (the drains keep ACT fed with the other mt's exps)
            pt_of = {}
            qk_chain(1, 0, xt_t[1])[1]()
            score_diag(1, 0, 500)
            qk_chain(1, 1, xt_t[1])[1]()
            score_diag(1, 1, 500)
            for ik in range(0, 4):
                for mt in range(2):
                    ptt = pt_pool.tile([P, 2, TB], BF16, tag="pt", name="pt")
                    pt_of[(1, mt, ik)] = ptt
                    score_off(1, mt, ik, ptt[:], 850)

            # PV1 + B1 first (frees ptd(1) for S(jq2)'s diag)
            for mt in range(2):
                queue_pv(1, mt,
                         lambda ik, hh, mt=mt: pt_of[(1, mt, ik)][:, hh, :])
            for t4 in range(NTB):
                pending.append(v_chain(2, t4, xt_t[2]))
            queue_proj(0)

            # S(jq3, k-chunks 4-7)
            for ik in range(4, 8):
                for mt in range(2):
                    score_off(3, mt, ik, j3pt[mt][:, ik, :, :], 900)

            # S(jq2): qk chains inline per mt
            qk_chain(2, 0, xt_t[2])[1]()
            score_diag(2, 0, 500)
            qk_chain(2, 1, xt_t[2])[1]()
            score_diag(2, 1, 500)
            for ik in range(0, 8):
                for mt in range(2):
                    ptt = pt_pool.tile([P, 2, TB], BF16, tag="pt", name="pt")
                    pt_of[(2, mt, ik)] = ptt
                    score_off(2, mt, ik, ptt[:], 900)

            # PV2 + B2 first (frees ptd(2) for S(jq3)'s diag)
            for mt in range(2):
                queue_pv(2, mt,
                         lambda ik, hh, mt=mt: pt_of[(2, mt, ik)][:, hh, :])
            for t4 in range(NTB):
                pending.append(v_chain(3, t4, xt3_sb))
            queue_proj(1)

            # S(jq3, k-chunks 8-11)
            for ik in range(8, 12):
                for mt in range(2):
                    score_off(3, mt, ik, j3pt[mt][:, ik, :, :], 1100)

            # S(jq3 diag): k3 chains inline per mt; last exps
            k_chain(3, 0, xt3_sb)[1]()
            score_diag(3, 0, 1400)
            k_chain(3, 1, xt3_sb)[1]()
            score_diag(3, 1, 1400)

            # PV3 full chains per pair, off part first (those exps are all
            # done, so the chains stream without ACT waits)
            for mt in range(2):
                queue_pv(3, mt,
                         lambda ik, hh, mt=mt: j3pt[mt][:, ik, hh, :],
                         part="all_off_first")
            queue_proj(2)
            flush_all()
            queue_proj(3)
            flush_all()

    nc.compile()
    _CACHE["nc"] = nc
    return nc


def _prep_core_inputs(x, w_attn, b_attn, w_proj, c):
    b, hg = divmod(c, 4)
    cs = slice(CL * hg, CL * (hg + 1))  # this core's 256 channels
    scale = np.float32(1.0 / np.sqrt(D))

    xt = np.ascontiguousarray(
        x[b].reshape(NTB, TB, NC_C, P).transpose(0, 3, 2, 1)).astype(NP_BF16)
    wq = np.ascontiguousarray(
        (w_attn[:, cs] * scale).reshape(NC_C, P, CL).transpose(1, 0, 2)
    ).astype(NP_BF16)
    wk = np.ascontiguousarray(
        w_attn[:, C:][:, cs].reshape(NC_C, P, CL).transpose(1, 0, 2)
    ).astype(NP_BF16)
    wv = np.ascontiguousarray(
        w_attn[:, 2 * C:][:, cs].reshape(NC_C, P, CL).transpose(1, 0, 2)
    ).astype(NP_BF16)
    bq = np.ascontiguousarray((b_attn[cs] * scale).reshape(2, P).T)
    bk = np.ascontiguousarray(b_attn[C:][cs].reshape(2, P).T)
    bv = np.ascontiguousarray(np.broadcast_to(b_attn[2 * C:][cs], (P, CL)))
    wp = np.ascontiguousarray(
        w_proj[cs, :].reshape(2, P, C).transpose(1, 0, 2)).astype(NP_BF16)

    p_idx = np.arange(P)[:, None, None, None]
    col = np.arange(P)[None, None, None, :]
    tri = np.ascontiguousarray(
        np.broadcast_to((col >= p_idx), (P, 4, 2, P))).astype(NP_BF16)

    return {"xt": xt, "wq": wq, "wk": wk, "wv": wv, "bq": bq, "bk": bk,
            "bv": bv, "wp": wp, "tri": tri}


def kernel(x, w_attn, b_attn, w_proj, b_proj):
    x = np.asarray(x, dtype=np.float32)
    w_attn = np.asarray(w_attn, dtype=np.float32)
    b_attn = np.asarray(b_attn, dtype=np.float32)
    w_proj = np.asarray(w_proj, dtype=np.float32)
    b_proj = np.asarray(b_proj, dtype=np.float32)

    nc = _build()
    in_maps = [_prep_core_inputs(x, w_attn, b_attn, w_proj, c)
               for c in range(N_CORES)]
    res = run_bass_kernel_spmd(nc, in_maps, list(range(N_CORES)))

    out = np.empty((B, T, C), dtype=np.float32)
    for b in range(B):
        acc = np.zeros((T, C), dtype=np.float32)
        for c in range(4 * b, 4 * b + 4):
            acc += res.results[c]["o"].astype(np.float32).reshape(T, C)
        out[b] = acc + b_proj
    return out


# revision 20
# speedup vs baseline: 1.0577x; 1.0002x over previous
"""Causal self-attention (B=2, T=2048, C=1024, H=16, D=64) on 8 TRN2 cores.

Sharding: data-parallel over batch (4 cores per batch element) x tensor-
parallel over heads (4 heads per core, as 2 pairs stacked on the 128
partitions). Per core: QKV projection for its head slice, causal attention in
a transposed dataflow (S^T kept as [k, q] so PV contracts over full
128-partition k chunks), row-parallel output projection; the 4 partial
projection outputs per batch are summed on the host, plus the bias.

Perf structure (v2):
- Score matmuls have K=D=64: the two heads of a pair are issued as two
  concurrent PE row-tiles (lhsT/rhs at base partitions 0/64 -> auto
  tile_position (0,0)/(64,0)), so scores run at full array rate.
- Diagonal k-chunks are N-restricted: chunk m of a q-block only computes
  columns >= 128m (the rest is fully masked). The exp output is stored
  column-shifted so the per-pair mask multiply is one [128,4,2,128] DVE op
  on the leading triangle.
- Softmax denominators ride as a ones-column in V (PV out M=65, free);
  reciprocal via reciprocal_approx_fast; the per-q recip row is broadcast
  across partitions with two concurrent rank-1 PE matmuls at tile positions
  (0,0)/(64,64).
- ACT (exp) is the co-critical engine (~82us of exp at 153G elem/s vs
  ~100us of PE work). The causal structure back-loads exp work, so q for
  block 3 is computed right after block 0's QKV and jq=3's off-diagonal
  scores are emitted early, unlocked k-block by k-block. A pending-thunk
  queue interleaves QKV/PV/proj matmuls between score pairs so the PE
  never idles while ACT chews exps. PV chains consume diagonal probs first
  so the shared diag tiles recycle quickly.
- All matmul operands bf16 (FWL), accumulation fp32 in PSUM; q weights and
  bias pre-scaled by 1/sqrt(D); no max-subtraction in softmax (scores are
  O(1) for this input scale); masked-out entries are multiplied by 0 after
  exp. Output partials stored bf16.
"""

import numpy as np
import ml_dtypes

import concourse.bass as bass
import concourse.mybir as mybir
import concourse.tile as tile
from concourse import bacc
from concourse.bass_utils import run_bass_kernel_spmd

# Problem shape (hardcoded per contract)
B, T, C, H, D = 2, 2048, 1024, 16, 64
N_CORES = 8
P = 128            # partitions
TB = 512           # q-block width
NTB = T // TB      # 4 q-blocks
NT = T // P        # 16 t-tiles
NC_C = C // P      # 8 contraction chunks over C
HL = 4             # heads per core
CL = HL * D        # 256 local channels
F32 = mybir.dt.float32
BF16 = mybir.dt.bfloat16
NP_BF16 = ml_dtypes.bfloat16
EXP = mybir.ActivationFunctionType.Exp
MUL = mybir.AluOpType.mult
ADD = mybir.AluOpType.add

_CACHE = {}


def _build():
    if "nc" in _CACHE:
        return _CACHE["nc"]
    nc = bacc.Bacc("TRN2", target_bir_lowering=False, debug=False,
                   num_devices=N_CORES)

    xt_d = nc.declare_dram_parameter("xt", [NTB, P, NC_C, TB], BF16, isOutput=False)
    wq_d = nc.declare_dram_parameter("wq", [P, NC_C, CL], BF16, isOutput=False)
    wk_d = nc.declare_dram_parameter("wk", [P, NC_C, CL], BF16, isOutput=False)
    wv_d = nc.declare_dram_parameter("wv", [P, NC_C, CL], BF16, isOutput=False)
    bq_d = nc.declare_dram_parameter("bq", [P, 2], F32, isOutput=False)
    bk_d = nc.declare_dram_parameter("bk", [P, 2], F32, isOutput=False)
    bv_d = nc.declare_dram_parameter("bv", [P, CL], F32, isOutput=False)
    wp_d = nc.declare_dram_parameter("wp", [P, 2, C], BF16, isOutput=False)
    tri_d = nc.declare_dram_parameter("tri", [P, 4, 2, P], BF16, isOutput=False)
    o_d = nc.declare_dram_parameter("o", [NT, P, C], BF16, isOutput=True)

    with tile.TileContext(nc) as tc:
        with (
            tc.tile_pool(name="const", bufs=1) as cw,
            tc.tile_pool(name="xt", bufs=2) as xt_pool,
            tc.tile_pool(name="qkv", bufs=1) as qkv_pool,
            tc.tile_pool(name="pt", bufs=20) as pt_pool,
            tc.tile_pool(name="ptd", bufs=3) as ptd_pool,
            tc.tile_pool(name="stage", bufs=3) as stage_pool,
            tc.tile_pool(name="pA", bufs=2, space="PSUM") as pA,   # 2 banks/tile
            tc.tile_pool(name="pB", bufs=3, space="PSUM") as pB,   # 1 bank/tile
            tc.tile_pool(name="pC", bufs=1, space="PSUM") as pC,   # 1 bank/tile
        ):
            # --- persistent SBUF tensors; DMA in priority order ---
            wq_sb = cw.tile([P, NC_C, CL], BF16)
            wk_sb = cw.tile([P, NC_C, CL], BF16)
            wv_sb = cw.tile([P, NC_C, CL], BF16)
            bq_sb = cw.tile([P, 2], F32)
            bk_sb = cw.tile([P, 2], F32)
            bv_sb = cw.tile([P, CL], F32)
            wp_sb = cw.tile([P, 2, C], BF16)
            tri_sb = cw.tile([P, 4, 2, P], BF16)
            xt3_sb = cw.tile([P, NC_C, TB], BF16)
            dum_sb = cw.tile([1, 8], F32)
            xt_t = [None] * 3

            nc.sync.dma_start(wq_sb[:], wq_d[:])
            xt_t[0] = xt_pool.tile([P, NC_C, TB], BF16, tag="xt", name="xt0")
            nc.sync.dma_start(xt_t[0][:], xt_d[0])
            nc.sync.dma_start(bq_sb[:], bq_d[:])
            nc.sync.dma_start(wk_sb[:], wk_d[:])
            nc.sync.dma_start(bk_sb[:], bk_d[:])
            nc.sync.dma_start(xt3_sb[:], xt_d[3])
            nc.sync.dma_start(wv_sb[:], wv_d[:])
            nc.sync.dma_start(bv_sb[:], bv_d[:])
            nc.sync.dma_start(tri_sb[:], tri_d[:])
            xt_t[1] = xt_pool.tile([P, NC_C, TB], BF16, tag="xt", name="xt1")
            nc.sync.dma_start(xt_t[1][:], xt_d[1])
            nc.sync.dma_start(wp_sb[:], wp_d[:])

            # preload the exp table set while DMAs stream
            nc.vector.memset(dum_sb[:], 1.0)
            nc.scalar.activation(dum_sb[:], dum_sb[:], EXP)

            # PE warm-up: dummy matmuls on memset data while the input DMAs
            # stream, so QKV0 starts at the un-throttled clock
            warm_sb = cw.tile([P, P], BF16)
            nc.vector.memset(warm_sb[:], 0.0)
            wps = pA.tile([P, 2, TB], F32, tag="pA", name="wps")
            for i in range(40):
                nc.tensor.matmul(wps[:, 0, 0:P], warm_sb[:], warm_sb[:],
                                 start=True, stop=True, skip_group_check=True)


            # qT/kT: [128 = 2 heads x 64d, T]; mt=0 -> heads 0,1; 1 -> 2,3
            q_sb = [qkv_pool.tile([P, T], BF16, tag=f"q{m}", name=f"q{m}")
                    for m in range(2)]
            k_sb = [qkv_pool.tile([P, T], BF16, tag=f"k{m}", name=f"k{m}")
                    for m in range(2)]
            a_sb = [qkv_pool.tile([P, T], BF16, tag=f"a{m}", name=f"a{m}")
                    for m in range(2)]
            # V (+ ones column): [p(k within chunk), t-tile, head, 65]
            v_sb = qkv_pool.tile([P, NT, HL, D + 1], BF16, tag="v")
            nc.vector.memset(v_sb[:, :, :, D:D + 1], 1.0)
            # jq=3 off-diagonal probs, alive until PV3: [chunk, hh, q]
            j3pt = [qkv_pool.tile([P, 12, 2, TB], BF16, tag=f"j3{m}",
                                  name=f"j3{m}") for m in range(2)]
            # rank-1 broadcast lhsT: ones at partition rows 0..64
            ones_sb = cw.tile([P // 2 + 1, D], F32)
            nc.vector.memset(ones_sb[:], 1.0)

            # ---------- pending-thunk machinery ----------
            pending = []      # list of (cost_ns, fn)
            n_popped = [0]

            def drain(budget_ns):
                while pending and budget_ns > 0:
                    cost, fn = pending.pop(0)
                    fn()
                    n_popped[0] += 1
                    budget_ns -= cost

            def mark():
                return n_popped[0] + len(pending)

            def flush_upto(m):
                while n_popped[0] < m:
                    _, fn = pending.pop(0)
                    fn()
                    n_popped[0] += 1

            def flush_all():
                while pending:
                    _, fn = pending.pop(0)
                    fn()
                    n_popped[0] += 1

            # ---------- QKV building blocks ----------
            def qk_chain(jt, mt, xsrc):
                def fn(jt=jt, mt=mt, xsrc=xsrc):
                    tsl = bass.ts(jt, TB)
                    msl = bass.ts(mt, P)
                    pqk = pA.tile([P, 2, TB], F32, tag="pA", name="pqk")
                    for c in range(NC_C):
                        nc.tensor.matmul(pqk[:, 0, :], wq_sb[:, c, msl],
                                         xsrc[:, c, :],
                                         start=(c == 0), stop=(c == NC_C - 1),
                                         skip_group_check=True)
                    for c in range(NC_C):
                        nc.tensor.matmul(pqk[:, 1, :], wk_sb[:, c, msl],
                                         xsrc[:, c, :],
                                         start=(c == 0), stop=(c == NC_C - 1),
                                         skip_group_check=True)
                    nc.vector.tensor_scalar_add(q_sb[mt][:, tsl], pqk[:, 0, :],
                                                bq_sb[:, mt:mt + 1])
                    nc.vector.tensor_scalar_add(k_sb[mt][:, tsl], pqk[:, 1, :],
                                                bk_sb[:, mt:mt + 1])
                return (3700, fn)

            def q_chain(jt, mt, xsrc):
                def fn(jt=jt, mt=mt, xsrc=xsrc):
                    pq = pA.tile([P, 2, TB], F32, tag="pA", name="pq")
                    for c in range(NC_C):
                        nc.tensor.matmul(pq[:, 0, :], wq_sb[:, c, bass.ts(mt, P)],
                                         xsrc[:, c, :],
                                         start=(c == 0), stop=(c == NC_C - 1),
                                         skip_group_check=True)
                    nc.vector.tensor_scalar_add(q_sb[mt][:, bass.ts(jt, TB)],
                                                pq[:, 0, :], bq_sb[:, mt:mt + 1])
                return (1850, fn)

            def k_chain(jt, mt, xsrc):
                def fn(jt=jt, mt=mt, xsrc=xsrc):
                    pk = pA.tile([P, 2, TB], F32, tag="pA", name="pk")
                    for c in range(NC_C):
                        nc.tensor.matmul(pk[:, 0, :], wk_sb[:, c, bass.ts(mt, P)],
                                         xsrc[:, c, :],
                                         start=(c == 0), stop=(c == NC_C - 1),
                                         skip_group_check=True)
                    nc.vector.tensor_scalar_add(k_sb[mt][:, bass.ts(jt, TB)],
                                                pk[:, 0, :], bk_sb[:, mt:mt + 1])
                return (1850, fn)

            def v_chain(jt, t4, xsrc):
                def fn(jt=jt, t4=t4, xsrc=xsrc):
                    tt = NTB * jt + t4
                    psv_t = pA.tile([P, 2, TB], F32, tag="pA", name="psv")
                    psv = psv_t[:, 0, 0:CL]
                    for c in range(NC_C):
                        nc.tensor.matmul(psv, xsrc[:, c, bass.ts(t4, P)],
                                         wv_sb[:, c, :],
                                         start=(c == 0), stop=(c == NC_C - 1),
                                         skip_group_check=True)
                    nc.vector.tensor_tensor(
                        v_sb[:, tt, :, 0:D],
                        psv.rearrange("p (h d) -> p h d", h=HL),
                        bv_sb[:].rearrange("p (h d) -> p h d", h=HL),
                        ADD)
                return (2050, fn)

            # ---------- attention building blocks ----------
            ptd_of = {}

            def score_diag(jq, mt, budget):
                """4 diagonal chunk-pairs + triangle mask for (jq, mt).
                Chunk m computes cols [128m, 512); exp stored column-shifted
                (ptd col c = q - 128m) so the mask is one leading-triangle
                multiply."""
                ptd = ptd_pool.tile([P, NTB, 2, TB], BF16, tag="ptd",
                                    name="ptd")
                ptd_of[(jq, mt)] = ptd
                for m in range(NTB):
                    ik = NTB * jq + m
                    w = TB - P * m
                    ps2 = pA.tile([P, 2, TB], F32, tag="pA", name="ps2")
                    for hh in range(2):
                        hsl = bass.ts(hh, D)
                        nc.tensor.matmul(
                            ps2[:, hh, P * m:TB],
                            k_sb[mt][hsl, bass.ts(ik, P)],
                            q_sb[mt][hsl, TB * jq + P * m:TB * (jq + 1)],
                            start=True, stop=True, skip_group_check=True)
                    nc.scalar.activation(ptd[:, m, :, 0:w],
                                         ps2[:, :, P * m:TB], EXP)
                    drain(budget)
                nc.vector.tensor_tensor(ptd[:, :, :, 0:P], ptd[:, :, :, 0:P],
                                        tri_sb[:], MUL)

            def score_off(jq, mt, ik, out_ap, budget):
                ps2 = pA.tile([P, 2, TB], F32, tag="pA", name="ps2")
                for hh in range(2):
                    hsl = bass.ts(hh, D)
                    nc.tensor.matmul(ps2[:, hh, :],
                                     k_sb[mt][hsl, bass.ts(ik, P)],
                                     q_sb[mt][hsl, bass.ts(jq, TB)],
                                     start=True, stop=True,
                                     skip_group_check=True)
                nc.scalar.activation(out_ap, ps2[:], EXP)
                drain(budget)

            pa_state = {}

            def queue_pv(jq, mt, pts_of, part="all"):
                """PV chains for both heads of pair mt. diag chunks first
                (frees the shared ptd tile early). part="off"/"diag" splits
                jq3's chain so the long off part pre-drains as soon as its
                exps exist, before the diagonal exps are even emitted."""
                pa2 = pa_state.setdefault((jq, mt), [None, None])
                offs = list(range(NTB * jq))
                order = []
                if part == "all_off_first":
                    order += [("o", ik) for ik in offs]
                    order += [("d", m) for m in range(NTB)]
                    starts, ends = ("o", 0), order[-1]
                elif part in ("all",):
                    order += [("d", m) for m in range(NTB)]
                    order += [("o", ik) for ik in offs]
                    starts, ends = ("d", 0), order[-1]
                elif part == "off":
                    order += [("o", ik) for ik in offs]
                    starts, ends = ("o", 0), (None, None)
                else:  # diag tail after an off part
                    order += [("d", m) for m in range(NTB)]
                    starts, ends = (None, None), ("d", NTB - 1)
                for i, (kind, idx) in enumerate(order):
                    first = (kind, idx) == starts
                    last = (kind, idx) == ends
                    for hh in range(2):
                        if kind == "d":
                            m = idx
                            def mm(jq=jq, mt=mt, m=m, hh=hh, w=TB - P * m,
                                   first=first, last=last):
                                if first:
                                    pa2[hh] = pB.tile([D + 1, TB], F32,
                                                      tag="pB", name="pa")
                                nc.tensor.matmul(pa2[hh][:, TB - w:TB],
                                                 v_sb[:, NTB * jq + m,
                                                      2 * mt + hh, :],
                                                 ptd_of[(jq, mt)][:, m, hh, 0:w],
                                                 start=first, stop=last,
                                                 skip_group_check=True)
                        else:
                            ik = idx
                            def mm(mt=mt, ik=ik, hh=hh, pts_of=pts_of,
                                   first=first, last=last):
                                if first:
                                    pa2[hh] = pB.tile([D + 1, TB], F32,
                                                      tag="pB", name="pa")
                                nc.tensor.matmul(pa2[hh][:],
                                                 v_sb[:, ik, 2 * mt + hh, :],
                                                 pts_of(ik, hh),
                                                 start=first, stop=last,
                                                 skip_group_check=True)
                        pending.append((430, mm))

                def norm(jq=jq, mt=mt):
                    qsl = bass.ts(jq, TB)
                    ua = stage_pool.tile([P, TB], F32, tag="ua", name="ua",
                                         bufs=2)
                    dn = stage_pool.tile([P // 2 + 1, TB], F32, tag="dn",
                                         name="dn", bufs=2)
                    rc = stage_pool.tile([P // 2 + 1, TB], F32, tag="rc",
                                         name="rc", bufs=2)
                    for hh in range(2):
                        nc.vector.tensor_copy(ua[64 * hh:64 * hh + D, :],
                                              pa2[hh][0:D, :])
                        nc.vector.tensor_copy(dn[64 * hh:64 * hh + 1, :],
                                              pa2[hh][D:D + 1, :])
                    nc.vector.reciprocal_approx_fast(rc[:], dn[:])
                    bcp_t = pC.tile([P, TB], F32, tag="pC", name="bcp")
                    bcp = bcp_t[:]
                    for hh in range(2):
                        nc.tensor.matmul(bcp[64 * hh:64 * hh + D, :],
                                         ones_sb[64 * hh:64 * hh + 1, :],
                                         rc[64 * hh:64 * hh + 1, :],
                                         start=True, stop=True,
                                         skip_group_check=True)
                    nc.vector.tensor_tensor(a_sb[mt][:, qsl], ua[:], bcp,
                                            MUL)
                if part != "off":
                    pending.append((900, norm))

            def queue_proj(jq):
                for t4 in range(NTB):
                    def fn(jq=jq, t4=t4):
                        tt = NTB * jq + t4
                        pso = pA.tile([P, 2, TB], F32, tag="pA", name="pso")
                        for nt in range(2):
                            for c2 in range(2):
                                nc.tensor.matmul(
                                    pso[:, nt, :],
                                    a_sb[c2][:, bass.ts(tt, P)],
                                    wp_sb[:, c2, bass.ts(nt, TB)],
                                    start=(c2 == 0), stop=(c2 == 1),
                                    skip_group_check=True)
                        st = stage_pool.tile([P, 2 * TB], BF16, tag="st",
                                             name="st", bufs=2)
                        if t4 % 2 == 0:
                            nc.vector.tensor_copy(
                                st[:].rearrange("p (n f) -> p n f", n=2),
                                pso[:])
                        else:
                            nc.scalar.activation(
                                st[:].rearrange("p (n f) -> p n f", n=2),
                                pso[:], mybir.ActivationFunctionType.Copy)
                        nc.sync.dma_start(o_d[tt], st[:])
                    pending.append((1900, fn))

            # ================= emission =================
            # First qk chain inline, then scores start immediately so ACT
            # (the co-critical engine) ramps as early as possible; the rest
            # of block 0 + q3 drain as fillers between ACT-paced pairs.
            qk_chain(0, 0, xt_t[0])[1]()
            pending.append(qk_chain(0, 1, xt_t[0]))
            for t4 in range(NTB):
                pending.append(v_chain(0, t4, xt_t[0]))
            pending.append(q_chain(3, 0, xt3_sb))
            pending.append(q_chain(3, 1, xt3_sb))

            score_diag(0, 0, 900)
            score_diag(0, 1, 900)
            flush_all()

            # PV0 + B0 first: PV's diag part frees ptd(0) tiles, which
            # S(jq1)'s diagonal exps need for their ptd allocations
            for mt in range(2):
                queue_pv(0, mt, None)
            for mt in range(2):
                pending.append(qk_chain(1, mt, xt_t[1]))
            m_qk1 = mark()
            for t4 in range(NTB):
                pending.append(v_chain(1, t4, xt_t[1]))

            # S(jq3, k-chunks 0-3)
            for ik in range(0, 4):
                for mt in range(2):
                    score_off(3, mt, ik, j3pt[mt][:, ik, :, :], 900)

            flush_upto(m_qk1)
            # prefetch xt2 (slot freed once xt0 consumed)
            xt_t[2] = xt_pool.tile([P, NC_C, TB], BF16, tag="xt", name="xt2")
            nc.sync.dma_start(xt_t[2][:], xt_d[2])

            # S(jq1): diag + off 0-3
            pt_of = {}
            for mt in range(2):
                score_diag(1, mt, 850)
            for ik in range(0, 4):
                for mt in range(2):
                    ptt = pt_pool.tile([P, 2, TB], BF16, tag="pt", name="pt")
                    pt_of[(1, mt, ik)] = ptt
                    score_off(1, mt, ik, ptt[:], 850)

            # PV1 + B1 first (frees ptd(1) for S(jq2)'s diag)
            for mt in range(2):
                queue_pv(1, mt,
                         lambda ik, hh, mt=mt: pt_of[(1, mt, ik)][:, hh, :])
            for mt in range(2):
                pending.append(qk_chain(2, mt, xt_t[2]))
            m_qk2 = mark()
            for t4 in range(NTB):
                pending.append(v_chain(2, t4, xt_t[2]))
            queue_proj(0)

            # S(jq3, k-chunks 4-7)
            for ik in range(4, 8):
                for mt in range(2):
                    score_off(3, mt, ik, j3pt[mt][:, ik, :, :], 900)

            flush_upto(m_qk2)

            # S(jq2): diag + off 0-7
            for mt in range(2):
                score_diag(2, mt, 900)
            for ik in range(0, 8):
                for mt in range(2):
                    ptt = pt_pool.tile([P, 2, TB], BF16, tag="pt", name="pt")
                    pt_of[(2, mt, ik)] = ptt
                    score_off(2, mt, ik, ptt[:], 900)

            # PV2 + B2 first (frees ptd(2) for S(jq3)'s diag)
            for mt in range(2):
                queue_pv(2, mt,
                         lambda ik, hh, mt=mt: pt_of[(2, mt, ik)][:, hh, :])
            for mt in range(2):
                pending.append(k_chain(3, mt, xt3_sb))
            m_k3 = mark()
            for t4 in range(NTB):
                pending.append(v_chain(3, t4, xt3_sb))
            queue_proj(1)

            # S(jq3, k-chunks 8-11)
            for ik in range(8, 12):
                for mt in range(2):
                    score_off(3, mt, ik, j3pt[mt][:, ik, :, :], 1100)

            flush_upto(m_k3)

            # S(jq3 diag): last exps; everything else drains behind them
            for mt in range(2):
                score_diag(3, mt, 1400)

            # PV3 full chains per pair, off part first (those exps are all
            # done, so the chains stream without ACT waits)
            for mt in range(2):
                queue_pv(3, mt,
                         lambda ik, hh, mt=mt: j3pt[mt][:, ik, hh, :],
                         part="all_off_first")
            queue_proj(2)
            flush_all()
            queue_proj(3)
            flush_all()

    nc.compile()
    _CACHE["nc"] = nc
    return nc


def _prep_core_inputs(x, w_attn, b_attn, w_proj, c):
    b, hg = divmod(c, 4)
    cs = slice(CL * hg, CL * (hg + 1))  # this core's 256 channels
    scale = np.float32(1.0 / np.sqrt(D))

    xt = np.ascontiguousarray(
        x[b].reshape(NTB, TB, NC_C, P).transpose(0, 3, 2, 1)).astype(NP_BF16)
    wq = np.ascontiguousarray(
        (w_attn[:, cs] * scale).reshape(NC_C, P, CL).transpose(1, 0, 2)
    ).astype(NP_BF16)
    wk = np.ascontiguousarray(
        w_attn[:, C:][:, cs].reshape(NC_C, P, CL).transpose(1, 0, 2)
    ).astype(NP_BF16)
    wv = np.ascontiguousarray(
        w_attn[:, 2 * C:][:, cs].reshape(NC_C, P, CL).transpose(1, 0, 2)
    ).astype(NP_BF16)
    bq = np.ascontiguousarray((b_attn[cs] * scale).reshape(2, P).T)
    bk = np.ascontiguousarray(b_attn[C:][cs].reshape(2, P).T)
    bv = np.ascontiguousarray(np.broadcast_to(b_attn[2 * C:][cs], (P, CL)))
    wp = np.ascontiguousarray(
        w_proj[cs, :].reshape(2, P, C).transpose(1, 0, 2)).astype(NP_BF16)

    p_idx = np.arange(P)[:, None, None, None]
    col = np.arange(P)[None, None, None, :]
    tri = np.ascontiguousarray(
        np.broadcast_to((col >= p_idx), (P, 4, 2, P))).astype(NP_BF16)

    return {"xt": xt, "wq": wq, "wk": wk, "wv": wv, "bq": bq, "bk": bk,
            "bv": bv, "wp": wp, "tri": tri}


def kernel(x, w_attn, b_attn, w_proj, b_proj):
    x = np.asarray(x, dtype=np.float32)
    w_attn = np.asarray(w_attn, dtype=np.float32)
    b_attn = np.asarray(b_attn, dtype=np.float32)
    w_proj = np.asarray(w_proj, dtype=np.float32)
    b_proj = np.asarray(b_proj, dtype=np.float32)

    nc = _build()
    in_maps = [_prep_core_inputs(x, w_attn, b_attn, w_proj, c)
               for c in range(N_CORES)]
    res = run_bass_kernel_spmd(nc, in_maps, list(range(N_CORES)))

    out = np.empty((B, T, C), dtype=np.float32)
    for b in range(B):
        acc = np.zeros((T, C), dtype=np.float32)
        for c in range(4 * b, 4 * b + 4):
            acc += res.results[c]["o"].astype(np.float32).reshape(T, C)
        out[b] = acc + b_proj
    return out
